# revision 8
# baseline (speedup 1.0000x reference)
"""Trainium2 Bass kernel for a rate-1/2, constraint-length-3 feedforward
convolutional encoder (generator polynomials "101" and "111", MSB-first).

The trellis scan in the reference collapses to elementwise XORs of shifted
input bits (zero initial state):

    out0[t] = u[t] ^ u[t-2]            (poly "101")
    out1[t] = u[t] ^ u[t-1] ^ u[t-2]   (poly "111")

with the codeword interleaved time-major: y[:, 2t] = out0[t], y[:, 2t+1] = out1[t].

XOR on {0,1} floats is computed arithmetically: x ^ y = (x - y)^2.

Sharding: pure data parallel over the batch dim across 8 NeuronCores.
The kernel is DMA-bound (3 MiB of HBM traffic per 1 MiB of input); the
compute (2 vector + 2 scalar ops per tile) hides entirely under the DMA.
"""

import numpy as np

N_CORES = 8
B, K = 8192, 2048
N_OUT = 2
SHARD_B = B // N_CORES  # 1024 codewords per core
P = 128                 # SBUF partitions
HK = K // 2             # column half processed per iteration

_compiled = {}


def _build_nc():
    import concourse.bass as bass  # noqa: F401
    import concourse.tile as tile
    from concourse import bacc, mybir

    nc = bacc.Bacc(
        "TRN2",
        target_bir_lowering=False,
        debug=False,
        enable_asserts=False,
    )
    x = nc.dram_tensor("x", [SHARD_B, K], mybir.dt.float32, kind="ExternalInput").ap()
    y = nc.dram_tensor(
        "y", [SHARD_B, N_OUT * K], mybir.dt.float32, kind="ExternalOutput"
    ).ap()

    # 256-row blocks: partition p holds rows (2p, 2p+1) of the block, so
    # each partition's DMA chunk is 16 KiB contiguous DRAM on the input and
    # 32 KiB contiguous on the output — bigger packets, higher SDMA rate.
    RPB = 2 * P  # rows per block
    n_blocks = SHARD_B // RPB  # 4
    SEG = K + 2  # one codeword segment incl. 2 zero columns
    N_SLOTS = 3

    with tile.TileContext(nc) as tc:
        with (
            tc.tile_pool(name="xin", bufs=1) as in_pool,
            tc.tile_pool(name="out", bufs=3) as out_pool,
            tc.tile_pool(name="tmp", bufs=3) as tmp_pool,
        ):
            # Persistent input slots; each holds two codewords per partition,
            # each codeword prefixed by 2 zero columns so the shifted views
            # u[t-1], u[t-2] fall out of plain column offsets. The zero
            # columns are written ONCE here; the per-block DMAs only write
            # the data ranges, so no DMA ever waits on a memset.
            in_slots = [
                in_pool.tile(
                    [P, 2 * SEG], mybir.dt.float32, tag=f"xin{j}", name=f"xin{j}"
                )
                for j in range(N_SLOTS)
            ]
            for j in range(N_SLOTS):
                nc.vector.memset(in_slots[j][:, 0:2], 0.0)
                nc.vector.memset(in_slots[j][:, SEG : SEG + 2], 0.0)

            for blk in range(n_blocks):
                xin = in_slots[blk % N_SLOTS]
                r0 = blk * RPB
                # [256, K] -> [128, 2, K]: partition p <-> rows (2p, 2p+1)
                x_blk = x[r0 : r0 + RPB, :].rearrange("(p two) k -> p two k", two=2)
                y_blk = y[r0 : r0 + RPB, :].rearrange("(p two) k -> p two k", two=2)
                xin3 = xin.rearrange("p (two s) -> p two s", two=2)

                # One 2 MiB input DMA on the SP HWDGE ring (Sync sequencer);
                # DRAM side is 16 KiB contiguous per partition.
                nc.sync.dma_start(xin3[:, :, 2:SEG], x_blk)

                out = out_pool.tile(
                    [P, 2 * N_OUT * K], mybir.dt.float32, tag="out", name="out"
                )

                for s in range(2):
                    base = s * SEG
                    a = xin[:, base + 2 : base + SEG]      # u[t]
                    b = xin[:, base + 1 : base + 1 + K]    # u[t-1]
                    c = xin[:, base : base + K]            # u[t-2]
                    obase = s * N_OUT * K
                    even = out[:, obase : obase + N_OUT * K : 2]
                    odd = out[:, obase + 1 : obase + N_OUT * K : 2]

                    # p = a - c in {-1,0,1}; out0 = p^2 = a ^ c
                    p = tmp_pool.tile([P, K], mybir.dt.float32, tag="p", name="p")
                    nc.vector.tensor_tensor(p[:], a, c, mybir.AluOpType.subtract)
                    nc.scalar.square(even, p[:])

                    # q = out0 - b in {-1,0,1}; out1 = q^2 = out0 ^ b
                    # (reuses p's buffer: p is dead once the first square ran)
                    nc.vector.tensor_tensor(p[:], even, b, mybir.AluOpType.subtract)
                    nc.scalar.square(odd, p[:])

                # One 4 MiB output DMA on the ACT HWDGE ring (Scalar
                # sequencer) — 32 KiB contiguous per partition on both the
                # SBUF and DRAM sides, and a separate ring from the input
                # stream so neither stream's stalled trigger blocks the
                # other. The trigger lands right after the square that
                # finishes the tile, so it never stalls the ACT sequencer.
                nc.scalar.dma_start(y_blk, out.rearrange("p (two k) -> p two k", two=2))

    nc.compile()
    return nc


def _get_nc():
    if "nc" not in _compiled:
        _compiled["nc"] = _build_nc()
    return _compiled["nc"]


def kernel(**inputs) -> np.ndarray:
    from concourse.bass_utils import run_bass_kernel_spmd

    x_full = np.ascontiguousarray(np.asarray(inputs["inputs"], dtype=np.float32))
    assert x_full.shape == (B, K), x_full.shape

    nc = _get_nc()
    in_maps = [
        {"x": x_full[i * SHARD_B : (i + 1) * SHARD_B]} for i in range(N_CORES)
    ]
    res = run_bass_kernel_spmd(nc, in_maps, core_ids=list(range(N_CORES)))
    out = np.concatenate([r["y"] for r in res.results], axis=0)
    return np.ascontiguousarray(out, dtype=np.float32)
